# revision 16
# baseline (speedup 1.0000x reference)
"""Trainium2 Bass kernel for GraphormerAttention.

Problem: B=8, T=1024, C=512, H=8, D=64.
  q = x @ Wq.T + bq ; k = x @ Wk.T + bk ; v = x @ Wv.T + bv
  scores = einsum('bqhd,bkhd->bhqk', q, k) / sqrt(D) + attn_bias
  scores masked at key_padding_mask -> softmax -> out = attn @ v @ Wo.T + bo

Sharding: data-parallel over B across the 8 NeuronCores (1 batch each).

Device-side dataflow (all matmuls in bf16, fp32 PSUM accumulation):
  - Host pre-transposes x -> xT [C,T], weights -> W.T [c_in, c_out], and
    attn_bias -> exp(bias^T) [H, tk, tq] in bf16 with masked keys set to 0.
    The 1/sqrt(D) scale is folded into Wq. exp(S+bias) = exp(S)*exp(bias),
    so the device never adds the bias: ACT computes exp(S) straight out of
    PSUM and DVE multiplies by the preloaded exp(bias^T) tile.
  - Scores are computed transposed (S^T[tk, tq] = K_h @ Q_h^T) so that the
    attn @ V contraction (over tk) runs directly on the tk-partitioned P^T
    tiles with zero on-device transposes.
  - Softmax denominators come for free from the attn@V matmul: V is stored
    with a ones-column appended per head, so row D of the PSUM output is
    sum_tk P^T[tk, tq].

Schedule: 5-phase software pipeline keyed on head PAIRS (even head on PE
rows 0-63, odd head on rows 64-127):
  - Scores matmuls for a pair are emitted adjacently; their K=64
    contractions land on disjoint PE row-groups (tile_position derived from
    base partitions), so the hardware runs them concurrently (~2x).
  - Phase j runs scores(pair j) interleaved per tk-chunk with
    attnV(pair j-1), keeping the PE stream dense (HAM clock gate stays
    released). QKV projections fill phase 0's spare PE slots; the output
    projection runs as wave A (3/4 chunks) inside phase 4 and a short
    wave B tail.
  - Normalization for pairs 0-2: DVE copies the [65,512] attnV accumulator
    to SBUF bf16, the 4 denominator rows are DMA-folded through DRAM to a
    [128,16] tile, one cheap DVE reciprocal, DMA-broadcast back to [64,512]
    bf16 so the normalize multiplies run in the DVE 2x perf mode. Pair 3
    takes a low-latency DMA-free path instead (fp32 copy,
    reciprocal_approx_fast on the row, K=1 PE broadcast matmul) so the
    tail is not gated by the DMA round-trip.
"""

import math
import sys
from contextlib import ExitStack

import numpy as np

if "/opt/trn_rl_repo" not in sys.path:
    sys.path.insert(0, "/opt/trn_rl_repo")

import ml_dtypes

import concourse.bass as bass
import concourse.mybir as mybir
import concourse.tile as tile
from concourse import bacc
from concourse.bass_utils import run_bass_kernel_spmd

B, T, C, H = 8, 1024, 512, 8
D = C // H            # 64
NCORES = 8
KC = C // 128         # 4 contraction chunks of 128 over c
MT = T // 128         # 8 tiles of 128 over t
HALF = 512            # free-dim tile width (PSUM bank = 512 fp32)
NH = T // HALF        # 2
NP = H // 2           # 4 head pairs

BF = mybir.dt.bfloat16
F32 = mybir.dt.float32
BF_NP = ml_dtypes.bfloat16


def _bcast_ap(row_ap, parts):
    """AP view broadcasting a [1, N] AP across `parts` partitions."""
    return bass.AP(
        tensor=row_ap.tensor,
        offset=row_ap.offset,
        ap=[[0, parts]] + [list(d) for d in row_ap.ap[1:]],
    )


def _fold_ap(row_ap, parts):
    """View a [1, N] DRAM AP as [parts, N // parts]."""
    n = row_ap.ap[-1][1]
    f = n // parts
    return bass.AP(
        tensor=row_ap.tensor, offset=row_ap.offset, ap=[[f, parts], [1, f]]
    )


def _body(ctx, tc, xT, wqT, wkT, wvT, woT, ebT, bvec, out):
    nc = tc.nc

    const = ctx.enter_context(tc.tile_pool(name="const", bufs=1))
    ptp = ctx.enter_context(tc.tile_pool(name="ptp", bufs=16))
    ebp = ctx.enter_context(tc.tile_pool(name="ebp", bufs=6))
    avcp = ctx.enter_context(tc.tile_pool(name="avcp", bufs=6))
    rbp = ctx.enter_context(tc.tile_pool(name="rbp", bufs=4))
    prtp = ctx.enter_context(tc.tile_pool(name="prtp", bufs=8))
    sml = ctx.enter_context(tc.tile_pool(name="sml", bufs=3))
    scp = ctx.enter_context(tc.tile_pool(name="scp", bufs=2, space="PSUM"))
    app = ctx.enter_context(tc.tile_pool(name="app", bufs=4, space="PSUM"))
    scrp = ctx.enter_context(tc.tile_pool(name="scrp", bufs=4, space="DRAM"))

    # ---- constants: host pre-packs SBUF layouts so each tensor is ONE
    # DMA with large (4-8KB) contiguous descriptors; ordered so the Q/K
    # projections can start as early as possible ----
    x_s = const.tile([128, KC, T], BF, tag="x_s")
    nc.sync.dma_start(out=x_s, in_=xT)
    w_s = {}
    for name, w in (("q", wqT), ("k", wkT), ("v", wvT), ("o", woT)):
        w_s[name] = const.tile([128, KC, C], BF, tag=f"w{name}", name=f"w{name}_s")
    nc.sync.dma_start(out=w_s["q"], in_=wqT)
    nc.sync.dma_start(out=w_s["k"], in_=wkT)
    nc.sync.dma_start(out=w_s["v"], in_=wvT)
    # bq' and bk as per-partition scalars per co-chunk
    bqk_s = const.tile([128, 2, KC], F32, tag="bqk")
    nc.sync.dma_start(out=bqk_s, in_=bvec[0:2, :].rearrange("n (kc p) -> p n kc", p=128))
    # bv and bo broadcast along partitions (vary along the free co dim)
    bv_bc = const.tile([128, C], F32, tag="bv_bc")
    nc.sync.dma_start(out=bv_bc, in_=_bcast_ap(bvec[2:3, :], 128))
    nc.sync.dma_start(out=w_s["o"], in_=woT)
    bo_bc = const.tile([128, C], F32, tag="bo_bc")
    nc.sync.dma_start(out=bo_bc, in_=_bcast_ap(bvec[3:4, :], 128))

    # ones row at partition D feeds pair-3's K=1 broadcast matmul
    ones_t = const.tile([D + 1, D], BF, tag="ones_t")
    nc.gpsimd.memset(ones_t, 1.0)

    # ---- PE warmup: dense dummy matmuls during the constant-load window
    # release the HAM clock gate (~3.4us of activity) so the projections
    # run at 2.4 GHz as soon as their data lands ----
    warm_rhs = const.tile([D, HALF], BF, tag="warm_rhs")
    nc.gpsimd.memset(warm_rhs, 0.0)
    warm_ps = app.tile([D, HALF], F32, tag="ap", name="warm_ps")
    for _ in range(10):
        nc.tensor.matmul(
            warm_ps, ones_t[0:D, :], warm_rhs, start=True, stop=True
        )

    # V in natural [t, c] layout with a ones column per head
    v_ext = const.tile([128, MT, H, D + 1], BF, tag="v_ext")
    nc.gpsimd.memset(v_ext[:, :, :, D:D + 1], 1.0)

    # Q^T, K^T projections and attn output, [co, t] layout chunked over co
    q_s = const.tile([128, KC, T], BF, tag="q_s")
    k_s = const.tile([128, KC, T], BF, tag="k_s")
    ao_s = const.tile([128, KC, T], BF, tag="ao_s")

    def qkproj(m):
        for ws, dst, brow in ((w_s["q"], q_s, 0), (w_s["k"], k_s, 1)):
            for nh in range(NH):
                ps = app.tile([128, HALF], F32, tag="ap", name=f"qk{m}")
                for kc in range(KC):
                    nc.tensor.matmul(
                        ps,
                        ws[:, kc, m * 128:(m + 1) * 128],
                        x_s[:, kc, nh * HALF:(nh + 1) * HALF],
                        start=(kc == 0),
                        stop=(kc == KC - 1),
                    )
                nc.vector.tensor_scalar_add(
                    dst[:, m, nh * HALF:(nh + 1) * HALF], ps, bqk_s[:, brow, m:m + 1]
                )

    def vproj(t_i):
        ps = app.tile([128, C], F32, tag="ap", name=f"v{t_i}")
        for kc in range(KC):
            nc.tensor.matmul(
                ps,
                x_s[:, kc, t_i * 128:(t_i + 1) * 128],
                w_s["v"][:, kc, :],
                start=(kc == 0),
                stop=(kc == KC - 1),
            )
        nc.vector.tensor_add(
            v_ext[:, t_i, :, 0:D],
            ps[:].rearrange("p (h d) -> p h d", h=H),
            bv_bc[:].rearrange("p (h d) -> p h d", h=H),
        )

    # heads 0-1 need only co-chunks 0-1 of Q^T/K^T: start attention early
    qkproj(0)
    qkproj(1)

    # tk = mp*256 + half*128 + p
    ebr = ebT.rearrange("h (mp two p) t -> h mp p two t", two=2, p=128)

    pt_tiles = {}
    eb_tiles = {}
    avs = {}
    prt_tiles = []

    def norm_folded(g0, g1):
        """Normalize pair (g0, g1): bf16 copies, DRAM-folded reciprocal,
        DMA broadcast, 2x-mode multiplies."""
        avcs = {}
        scrd = scrp.tile([1, 4 * HALF], BF, tag="scrd", name=f"scrd{g0}")
        scrd2 = scrp.tile([1, 4 * HALF], BF, tag="scrd2", name=f"scrd2{g0}")
        for i, (h, nh) in enumerate(
            (h, nh) for h in (g0, g1) for nh in range(NH)
        ):
            avc = avcp.tile([D + 1, HALF], BF, tag="avc", name=f"avc{h}_{nh}")
            nc.vector.tensor_copy(avc, avs[(h, nh)])
            avcs[(h, nh)] = avc
            nc.sync.dma_start(
                out=scrd[:, i * HALF:(i + 1) * HALF], in_=avc[D:D + 1, :]
            )
        rc = sml.tile([128, 4 * HALF // 128], BF, tag="rc")
        nc.sync.dma_start(out=rc, in_=_fold_ap(scrd[:], 128))
        rc2 = sml.tile([128, 4 * HALF // 128], BF, tag="rc2")
        with nc.allow_low_precision(reason="softmax denom bf16"):
            nc.vector.reciprocal(rc2, rc)
        nc.sync.dma_start(out=_fold_ap(scrd2[:], 128), in_=rc2)
        for i, (h, nh) in enumerate(
            (h, nh) for h in (g0, g1) for nh in range(NH)
        ):
            hp = (h % 2) * D
            hc = h // 2
            rb = rbp.tile([D, HALF], BF, tag="rb", name=f"rb{h}_{nh}")
            nc.sync.dma_start(
                out=rb, in_=_bcast_ap(scrd2[:, i * HALF:(i + 1) * HALF], D)
            )
            nc.vector.tensor_mul(
                ao_s[hp:hp + D, hc, nh * HALF:(nh + 1) * HALF],
                avcs[(h, nh)][0:D, :],
                rb,
            )

    def norm_fast(g0, g1):
        """Normalize pair (g0, g1) with no DMA round-trip (short tail):
        fp32 copy, reciprocal_approx_fast on the denominator row, K=1 PE
        broadcast matmul, then multiply."""
        for h in (g0, g1):
            hp = (h % 2) * D
            hc = h // 2
            for nh in range(NH):
                avc = avcp.tile(
                    [D + 1, HALF], F32, tag="avc32", name=f"avf{h}_{nh}"
                )
                nc.vector.tensor_copy(avc, avs[(h, nh)])
                rcp = sml.tile([D + 1, HALF], F32, tag="rcp", name=f"rcp{h}_{nh}")
                nc.vector.reciprocal_approx_fast(
                    out=rcp[D:D + 1, :], in_=avc[D:D + 1, :]
                )
                rcb = sml.tile([D + 1, HALF], BF, tag="rcb", name=f"rcb{h}_{nh}")
                with nc.allow_low_precision(reason="softmax denom bf16"):
                    nc.vector.tensor_copy(rcb[D:D + 1, :], rcp[D:D + 1, :])
                rb = app.tile([D, HALF], F32, tag="ap", name=f"rbf{h}_{nh}")
                nc.tensor.matmul(
                    rb, ones_t[D:D + 1, :], rcb[D:D + 1, :], start=True, stop=True
                )
                nc.vector.tensor_mul(
                    ao_s[hp:hp + D, hc, nh * HALF:(nh + 1) * HALF],
                    avc[0:D, :],
                    rb,
                )

    for j in range(NP + 1):  # phases 0..4
        if j < NP:
            h0, h1 = 2 * j, 2 * j + 1
            for h in (h0, h1):
                for mp in range(MT // 2):
                    ebt = ebp.tile([128, 2, T], BF, tag="eb", name=f"eb{h}_{mp}")
                    nc.sync.dma_start(out=ebt, in_=ebr[h, mp, :, :, :])
                    eb_tiles[(h, mp)] = ebt
                    pt_tiles[(h, mp)] = ptp.tile(
                        [128, 2, T], BF, tag="pt", name=f"pt{h}_{mp}"
                    )
        if j >= 1:
            g0, g1 = 2 * (j - 1), 2 * (j - 1) + 1
            for h in (g0, g1):
                for nh in range(NH):
                    avs[(h, nh)] = app.tile(
                        [D + 1, HALF], F32, tag="ap", name=f"avs{h}_{nh}"
                    )

        for m in range(MT):
            if j < NP:
                mp, half = divmod(m, 2)
                sc = {}
                for h in (h0, h1):
                    sc[h] = scp.tile([128, T], F32, tag="scp", name=f"sc{h}")
                # nh-major, head-minor: adjacent matmuls hit disjoint PE
                # row-groups (rows 0-63 / 64-127) and run concurrently
                for nh in range(NH):
                    for h in (h0, h1):
                        hp = (h % 2) * D
                        hc = h // 2
                        nc.tensor.matmul(
                            sc[h][:, nh * HALF:(nh + 1) * HALF],
                            k_s[hp:hp + D, hc, m * 128:(m + 1) * 128],
                            q_s[hp:hp + D, hc, nh * HALF:(nh + 1) * HALF],
                            start=True,
                            stop=True,
                        )
            if j >= 1:
                for h in (g0, g1):
                    for nh in range(NH):
                        nc.tensor.matmul(
                            avs[(h, nh)],
                            v_ext[:, m, h, :],
                            pt_tiles[(h, m // 2)][:, m % 2, nh * HALF:(nh + 1) * HALF],
                            start=(m == 0),
                            stop=(m == MT - 1),
                        )
            if j == NP:
                # output projection wave A: chunks kc=0..2 (heads 0-5),
                # running inside phase 4's attnV stream on the freed scores
                # PSUM slots
                ps = scp.tile([128, C], F32, tag="scp", name=f"oA{m}")
                for kc in range(KC - 1):
                    nc.tensor.matmul(
                        ps,
                        ao_s[:, kc, m * 128:(m + 1) * 128],
                        w_s["o"][:, kc, :],
                        start=(kc == 0),
                        stop=(kc == KC - 2),
                    )
                prt = prtp.tile([128, C], F32, tag="prt")
                nc.vector.tensor_add(prt, ps, bo_bc)
                prt_tiles.append(prt)
            if j < NP:
                for h in (h0, h1):
                    nc.scalar.activation(
                        pt_tiles[(h, mp)][:, half, :],
                        sc[h],
                        mybir.ActivationFunctionType.Exp,
                    )
                if half == 1:
                    for h in (h0, h1):
                        # offload some eb multiplies to GpSimd (slow but
                        # otherwise idle); pair 3's must stay on DVE so the
                        # tail's attnV is not gated by a 4us GpSimd op
                        eng = nc.gpsimd if (mp == 3 and j < 3) else nc.vector
                        eng.tensor_mul(
                            pt_tiles[(h, mp)][:], pt_tiles[(h, mp)][:],
                            eb_tiles[(h, mp)][:],
                        )
            if j == 0:
                vproj(m)
                if m == 2:
                    qkproj(2)
                if m == 4:
                    qkproj(3)

        # ---- normalization of pair j-1 ----
        if j >= 1:
            norm_folded(g0, g1)

    # ---- output projection wave B: final chunk + partials, DMA out ----
    for t_i in range(MT):
        ps = app.tile([128, C], F32, tag="ap", name=f"oB{t_i}")
        nc.tensor.matmul(
            ps,
            ao_s[:, KC - 1, t_i * 128:(t_i + 1) * 128],
            w_s["o"][:, KC - 1, :],
            start=True,
            stop=True,
        )
        ot = sml.tile([128, C], F32, tag="ot")
        nc.vector.tensor_add(ot, ps, prt_tiles[t_i])
        nc.sync.dma_start(out=out[t_i * 128:(t_i + 1) * 128, :], in_=ot)


_CACHE = {}


def build_nc():
    if "nc" in _CACHE:
        return _CACHE["nc"]
    nc = bacc.Bacc(
        "TRN2", target_bir_lowering=False, debug=False, num_devices=NCORES
    )
    xT = nc.dram_tensor("xT", [128, KC, T], BF, kind="ExternalInput")
    wqT = nc.dram_tensor("wqT", [128, KC, C], BF, kind="ExternalInput")
    wkT = nc.dram_tensor("wkT", [128, KC, C], BF, kind="ExternalInput")
    wvT = nc.dram_tensor("wvT", [128, KC, C], BF, kind="ExternalInput")
    woT = nc.dram_tensor("woT", [128, KC, C], BF, kind="ExternalInput")
    ebT = nc.dram_tensor("ebT", [H, T, T], BF, kind="ExternalInput")
    bvec = nc.dram_tensor("bvec", [4, C], F32, kind="ExternalInput")
    out = nc.dram_tensor("out", [T, C], F32, kind="ExternalOutput")
    with tile.TileContext(nc) as tc:
        with ExitStack() as ctx:
            _body(ctx, tc, xT[:], wqT[:], wkT[:], wvT[:], woT[:], ebT[:], bvec[:], out[:])
    nc.compile()
    _CACHE["nc"] = nc
    return nc


def make_in_maps(inputs):
    x = np.asarray(inputs["x"], dtype=np.float32)
    attn_bias = np.asarray(inputs["attn_bias"], dtype=np.float32)
    mask = np.asarray(inputs["key_padding_mask"]).astype(bool)
    Wq = np.asarray(inputs["Wq"], dtype=np.float32)
    Wk = np.asarray(inputs["Wk"], dtype=np.float32)
    Wv = np.asarray(inputs["Wv"], dtype=np.float32)
    Wo = np.asarray(inputs["Wo"], dtype=np.float32)
    bq = np.asarray(inputs["bq"], dtype=np.float32)
    bk = np.asarray(inputs["bk"], dtype=np.float32)
    bv = np.asarray(inputs["bv"], dtype=np.float32)
    bo = np.asarray(inputs["bo"], dtype=np.float32)

    def pack(wT):
        # [C_in, C_out] -> SBUF image [128, KC, C_out] (partition p holds
        # rows {kc*128+p}) so the whole tensor is one DMA of 128
        # contiguous descriptors
        return np.ascontiguousarray(
            wT.reshape(KC, 128, wT.shape[1]).transpose(1, 0, 2)
        ).astype(BF_NP)

    scale = math.sqrt(D)
    wqT = pack((Wq / scale).T)
    wkT = pack(Wk.T)
    wvT = pack(Wv.T)
    woT = pack(Wo.T)
    bvec = np.stack([bq / scale, bk, bv, bo]).astype(np.float32)

    in_maps = []
    for b in range(B):
        xT = pack(x[b].T)
        ebT = np.exp(attn_bias[b].transpose(0, 2, 1))
        ebT[:, mask[b], :] = 0.0
        ebT = ebT.astype(BF_NP)
        in_maps.append(
            {
                "xT": xT,
                "wqT": wqT,
                "wkT": wkT,
                "wvT": wvT,
                "woT": woT,
                "ebT": ebT,
                "bvec": bvec,
            }
        )
    return in_maps


def run(inputs, trace=False):
    nc = build_nc()
    in_maps = make_in_maps(inputs)
    res = run_bass_kernel_spmd(nc, in_maps, list(range(NCORES)), trace=trace)
    out = np.stack(
        [np.asarray(res.results[i]["out"], dtype=np.float32) for i in range(B)], axis=0
    )
    return out, res.exec_time_ns


def kernel(**inputs):
    out, _ = run(inputs, trace=False)
    return out


# revision 23
# speedup vs baseline: 1.1416x; 1.1416x over previous
"""Trainium2 Bass kernel for GraphormerAttention.

Problem: B=8, T=1024, C=512, H=8, D=64.
  q = x @ Wq.T + bq ; k = x @ Wk.T + bk ; v = x @ Wv.T + bv
  scores = einsum('bqhd,bkhd->bhqk', q, k) / sqrt(D) + attn_bias
  scores masked at key_padding_mask -> softmax -> out = attn @ v @ Wo.T + bo

Sharding: data-parallel over B across the 8 NeuronCores (1 batch each).

Device-side dataflow (all matmuls in bf16, fp32 PSUM accumulation):
  - Host pre-transposes x -> xT [C,T], weights -> W.T [c_in, c_out], and
    attn_bias -> exp(bias^T) [H, tk, tq] in bf16 with masked keys set to 0.
    The 1/sqrt(D) scale is folded into Wq. exp(S+bias) = exp(S)*exp(bias),
    so the device never adds the bias: ACT computes exp(S) straight out of
    PSUM and DVE multiplies by the preloaded exp(bias^T) tile.
  - Scores are computed transposed (S^T[tk, tq] = K_h @ Q_h^T) so that the
    attn @ V contraction (over tk) runs directly on the tk-partitioned P^T
    tiles with zero on-device transposes.
  - Softmax denominators come for free from the attn@V matmul: V is stored
    with a ones-column appended per head, so row D of the PSUM output is
    sum_tk P^T[tk, tq].

Schedule: 5-phase software pipeline keyed on head PAIRS (even head on PE
rows 0-63, odd head on rows 64-127):
  - Scores matmuls for a pair are emitted adjacently; their K=64
    contractions land on disjoint PE row-groups (tile_position derived from
    base partitions), so the hardware runs them concurrently (~2x).
  - Phase j runs scores(pair j) interleaved per tk-chunk with
    attnV(pair j-1), keeping the PE stream dense (HAM clock gate stays
    released). QKV projections fill phase 0's spare PE slots; the output
    projection runs as wave A (3/4 chunks) inside phase 4 and a short
    wave B tail.
  - Normalization for pairs 0-2: DVE copies the [65,512] attnV accumulator
    to SBUF bf16, the 4 denominator rows are DMA-folded through DRAM to a
    [128,16] tile, one cheap DVE reciprocal, DMA-broadcast back to [64,512]
    bf16 so the normalize multiplies run in the DVE 2x perf mode. Pair 3
    takes a low-latency DMA-free path instead (fp32 copy,
    reciprocal_approx_fast on the row, K=1 PE broadcast matmul) so the
    tail is not gated by the DMA round-trip.
"""

import math
import sys
from contextlib import ExitStack

import numpy as np

if "/opt/trn_rl_repo" not in sys.path:
    sys.path.insert(0, "/opt/trn_rl_repo")

import ml_dtypes

import concourse.bass as bass
import concourse.mybir as mybir
import concourse.tile as tile
from concourse import bacc
from concourse.bass_utils import run_bass_kernel_spmd

B, T, C, H = 8, 1024, 512, 8
D = C // H            # 64
NCORES = 8
KC = C // 128         # 4 contraction chunks of 128 over c
MT = T // 128         # 8 tiles of 128 over t
HALF = 512            # free-dim tile width (PSUM bank = 512 fp32)
NH = T // HALF        # 2
NP = H // 2           # 4 head pairs

BF = mybir.dt.bfloat16
F32 = mybir.dt.float32
BF_NP = ml_dtypes.bfloat16


def _bcast_ap(row_ap, parts):
    """AP view broadcasting a [1, N] AP across `parts` partitions."""
    return bass.AP(
        tensor=row_ap.tensor,
        offset=row_ap.offset,
        ap=[[0, parts]] + [list(d) for d in row_ap.ap[1:]],
    )


def _fold_ap(row_ap, parts):
    """View a [1, N] DRAM AP as [parts, N // parts]."""
    n = row_ap.ap[-1][1]
    f = n // parts
    return bass.AP(
        tensor=row_ap.tensor, offset=row_ap.offset, ap=[[f, parts], [1, f]]
    )


def _body(ctx, tc, xT, wqT, wkT, wvT, woT, ebT, bvec, out):
    nc = tc.nc

    const = ctx.enter_context(tc.tile_pool(name="const", bufs=1))
    ptp = ctx.enter_context(tc.tile_pool(name="ptp", bufs=16))
    ebp = ctx.enter_context(tc.tile_pool(name="ebp", bufs=6))
    avcp = ctx.enter_context(tc.tile_pool(name="avcp", bufs=6))
    rbp = ctx.enter_context(tc.tile_pool(name="rbp", bufs=4))
    prtp = ctx.enter_context(tc.tile_pool(name="prtp", bufs=8))
    sml = ctx.enter_context(tc.tile_pool(name="sml", bufs=3))
    scp = ctx.enter_context(tc.tile_pool(name="scp", bufs=2, space="PSUM"))
    app = ctx.enter_context(tc.tile_pool(name="app", bufs=4, space="PSUM"))
    scrp = ctx.enter_context(tc.tile_pool(name="scrp", bufs=4, space="DRAM"))

    # ---- constants: host pre-packs SBUF layouts so each tensor is ONE
    # DMA with large (4-8KB) contiguous descriptors; ordered so the Q/K
    # projections can start as early as possible ----
    x_s = const.tile([128, KC, T], BF, tag="x_s")
    nc.sync.dma_start(out=x_s, in_=xT)
    w_s = {}
    for name, w in (("q", wqT), ("k", wkT), ("v", wvT), ("o", woT)):
        w_s[name] = const.tile([128, KC, C], BF, tag=f"w{name}", name=f"w{name}_s")
    nc.sync.dma_start(out=w_s["q"], in_=wqT)
    nc.sync.dma_start(out=w_s["k"], in_=wkT)
    nc.sync.dma_start(out=w_s["v"], in_=wvT)
    # bq' and bk as per-partition scalars per co-chunk
    bqk_s = const.tile([128, 2, KC], F32, tag="bqk")
    nc.sync.dma_start(out=bqk_s, in_=bvec[0:2, :].rearrange("n (kc p) -> p n kc", p=128))
    # bv and bo broadcast along partitions (vary along the free co dim)
    bv_bc = const.tile([128, C], F32, tag="bv_bc")
    nc.sync.dma_start(out=bv_bc, in_=_bcast_ap(bvec[2:3, :], 128))
    nc.sync.dma_start(out=w_s["o"], in_=woT)
    bo_bc = const.tile([128, C], F32, tag="bo_bc")
    nc.sync.dma_start(out=bo_bc, in_=_bcast_ap(bvec[3:4, :], 128))

    # ones row at partition D feeds pair-3's K=1 broadcast matmul
    ones_t = const.tile([D + 1, D], BF, tag="ones_t")
    nc.gpsimd.memset(ones_t, 1.0)

    # ---- PE warmup: dense dummy matmuls during the constant-load window
    # release the HAM clock gate (~3.4us of activity) so the projections
    # run at 2.4 GHz as soon as their data lands ----
    warm_rhs = const.tile([D, HALF], BF, tag="warm_rhs")
    nc.gpsimd.memset(warm_rhs, 0.0)
    warm_ps = app.tile([D, HALF], F32, tag="ap", name="warm_ps")
    for _ in range(10):
        nc.tensor.matmul(
            warm_ps, ones_t[0:D, :], warm_rhs, start=True, stop=True
        )

    # V in natural [t, c] layout with a ones column per head
    v_ext = const.tile([128, MT, H, D + 1], BF, tag="v_ext")
    nc.gpsimd.memset(v_ext[:, :, :, D:D + 1], 1.0)

    # Q^T, K^T projections and attn output, [co, t] layout chunked over co
    q_s = const.tile([128, KC, T], BF, tag="q_s")
    k_s = const.tile([128, KC, T], BF, tag="k_s")
    ao_s = const.tile([128, KC, T], BF, tag="ao_s")

    def qkproj(m):
        for ws, dst, brow in ((w_s["q"], q_s, 0), (w_s["k"], k_s, 1)):
            for nh in range(NH):
                ps = app.tile([128, HALF], F32, tag="ap", name=f"qk{m}")
                for kc in range(KC):
                    nc.tensor.matmul(
                        ps,
                        ws[:, kc, m * 128:(m + 1) * 128],
                        x_s[:, kc, nh * HALF:(nh + 1) * HALF],
                        start=(kc == 0),
                        stop=(kc == KC - 1),
                    )
                nc.vector.tensor_scalar_add(
                    dst[:, m, nh * HALF:(nh + 1) * HALF], ps, bqk_s[:, brow, m:m + 1]
                )

    def vproj(t_i):
        ps = app.tile([128, C], F32, tag="ap", name=f"v{t_i}")
        for kc in range(KC):
            nc.tensor.matmul(
                ps,
                x_s[:, kc, t_i * 128:(t_i + 1) * 128],
                w_s["v"][:, kc, :],
                start=(kc == 0),
                stop=(kc == KC - 1),
            )
        nc.vector.tensor_add(
            v_ext[:, t_i, :, 0:D],
            ps[:].rearrange("p (h d) -> p h d", h=H),
            bv_bc[:].rearrange("p (h d) -> p h d", h=H),
        )

    # heads 0-1 need only co-chunk 0 of Q^T/K^T: start attention early;
    # the other co-chunks are produced inside phase 0's spare PE slots
    qkproj(0)

    # tk = mp*256 + half*128 + p
    ebr = ebT.rearrange("h (mp two p) t -> h mp p two t", two=2, p=128)

    pt_tiles = {}
    eb_tiles = {}
    avs = {}
    prt_tiles = []

    def norm_folded(g0, g1):
        """Normalize pair (g0, g1): bf16 copies, DRAM-folded reciprocal,
        DMA broadcast, 2x-mode multiplies."""
        avcs = {}
        scrd = scrp.tile([1, 4 * HALF], BF, tag="scrd", name=f"scrd{g0}")
        scrd2 = scrp.tile([1, 4 * HALF], BF, tag="scrd2", name=f"scrd2{g0}")
        for i, (h, nh) in enumerate(
            (h, nh) for h in (g0, g1) for nh in range(NH)
        ):
            avc = avcp.tile([D + 1, HALF], BF, tag="avc", name=f"avc{h}_{nh}")
            nc.vector.tensor_copy(avc, avs[(h, nh)])
            avcs[(h, nh)] = avc
            nc.sync.dma_start(
                out=scrd[:, i * HALF:(i + 1) * HALF], in_=avc[D:D + 1, :]
            )
        rc = sml.tile([128, 4 * HALF // 128], BF, tag="rc")
        nc.sync.dma_start(out=rc, in_=_fold_ap(scrd[:], 128))
        rc2 = sml.tile([128, 4 * HALF // 128], BF, tag="rc2")
        with nc.allow_low_precision(reason="softmax denom bf16"):
            nc.vector.reciprocal(rc2, rc)
        nc.sync.dma_start(out=_fold_ap(scrd2[:], 128), in_=rc2)
        for i, (h, nh) in enumerate(
            (h, nh) for h in (g0, g1) for nh in range(NH)
        ):
            hp = (h % 2) * D
            hc = h // 2
            rb = rbp.tile([D, HALF], BF, tag="rb", name=f"rb{h}_{nh}")
            nc.sync.dma_start(
                out=rb, in_=_bcast_ap(scrd2[:, i * HALF:(i + 1) * HALF], D)
            )
            nc.vector.tensor_mul(
                ao_s[hp:hp + D, hc, nh * HALF:(nh + 1) * HALF],
                avcs[(h, nh)][0:D, :],
                rb,
            )

    def norm_tail(g0, g1):
        """Normalize the last pair with a shorter chain than norm_folded:
        the reciprocal row comes back as a single-row read feeding K=1 PE
        broadcast matmuls instead of 4 partition-broadcast DMAs."""
        keys = [(h, nh) for h in (g0, g1) for nh in range(NH)]
        avcs = {}
        scrd = scrp.tile([1, 4 * HALF], BF, tag="scrd", name="scrdT")
        scrd2 = scrp.tile([1, 4 * HALF], BF, tag="scrd2", name="scrd2T")
        for i, (h, nh) in enumerate(keys):
            avc = avcp.tile([D + 1, HALF], BF, tag="avc", name=f"avt{h}_{nh}")
            nc.vector.tensor_copy(avc, avs[(h, nh)])
            avcs[(h, nh)] = avc
            nc.sync.dma_start(
                out=scrd[:, i * HALF:(i + 1) * HALF], in_=avc[D:D + 1, :]
            )
        rcf = sml.tile([128, 4 * HALF // 128], BF, tag="rcf")
        nc.sync.dma_start(out=rcf, in_=_fold_ap(scrd[:], 128))
        rcq = sml.tile([128, 4 * HALF // 128], BF, tag="rcq")
        with nc.allow_low_precision(reason="softmax denom bf16"):
            nc.vector.reciprocal(rcq, rcf)
        nc.sync.dma_start(out=_fold_ap(scrd2[:], 128), in_=rcq)
        rcr = const.tile([D + 1, 4, HALF], BF, tag="rcr")
        nc.sync.dma_start(
            out=rcr[D:D + 1, :, :],
            in_=scrd2[:].rearrange("one (i t) -> one i t", i=4),
        )
        for i, (h, nh) in enumerate(keys):
            hp = (h % 2) * D
            hc = h // 2
            rb = app.tile([D, HALF], F32, tag="ap", name=f"rbt{h}_{nh}")
            nc.tensor.matmul(
                rb, ones_t[D:D + 1, :], rcr[D:D + 1, i, :], start=True, stop=True
            )
            nc.vector.tensor_mul(
                ao_s[hp:hp + D, hc, nh * HALF:(nh + 1) * HALF],
                avcs[(h, nh)][0:D, :],
                rb,
            )

    for j in range(NP + 1):  # phases 0..4
        if j < NP:
            h0, h1 = 2 * j, 2 * j + 1
            for h in (h0, h1):
                for mp in range(MT // 2):
                    ebt = ebp.tile([128, 2, T], BF, tag="eb", name=f"eb{h}_{mp}")
                    nc.sync.dma_start(out=ebt, in_=ebr[h, mp, :, :, :])
                    eb_tiles[(h, mp)] = ebt
                    pt_tiles[(h, mp)] = ptp.tile(
                        [128, 2, T], BF, tag="pt", name=f"pt{h}_{mp}"
                    )
        if j >= 1:
            g0, g1 = 2 * (j - 1), 2 * (j - 1) + 1
            for h in (g0, g1):
                for nh in range(NH):
                    avs[(h, nh)] = app.tile(
                        [D + 1, HALF], F32, tag="ap", name=f"avs{h}_{nh}"
                    )

        for m in range(MT):
            if j < NP:
                mp, half = divmod(m, 2)
                sc = {}
                for h in (h0, h1):
                    sc[h] = scp.tile([128, T], F32, tag="scp", name=f"sc{h}")
                # nh-major, head-minor: adjacent matmuls hit disjoint PE
                # row-groups (rows 0-63 / 64-127) and run concurrently
                for nh in range(NH):
                    for h in (h0, h1):
                        hp = (h % 2) * D
                        hc = h // 2
                        nc.tensor.matmul(
                            sc[h][:, nh * HALF:(nh + 1) * HALF],
                            k_s[hp:hp + D, hc, m * 128:(m + 1) * 128],
                            q_s[hp:hp + D, hc, nh * HALF:(nh + 1) * HALF],
                            start=True,
                            stop=True,
                        )
            if j >= 1:
                for h in (g0, g1):
                    for nh in range(NH):
                        nc.tensor.matmul(
                            avs[(h, nh)],
                            v_ext[:, m, h, :],
                            pt_tiles[(h, m // 2)][:, m % 2, nh * HALF:(nh + 1) * HALF],
                            start=(m == 0),
                            stop=(m == MT - 1),
                        )
            if j == NP:
                # output projection wave A: chunks kc=0,1 (pairs 0-1, whose
                # normalization finished by end of phase 3), running inside
                # phase 4's attnV stream on the freed scores PSUM slots
                ps = scp.tile([128, C], F32, tag="scp", name=f"oA{m}")
                for kc in range(2):
                    nc.tensor.matmul(
                        ps,
                        ao_s[:, kc, m * 128:(m + 1) * 128],
                        w_s["o"][:, kc, :],
                        start=(kc == 0),
                        stop=(kc == 1),
                    )
                prt = prtp.tile([128, C], F32, tag="prt")
                nc.vector.tensor_add(prt, ps, bo_bc)
                prt_tiles.append(prt)
            if j < NP:
                for h in (h0, h1):
                    nc.scalar.activation(
                        pt_tiles[(h, mp)][:, half, :],
                        sc[h],
                        mybir.ActivationFunctionType.Exp,
                    )
                if half == 1:
                    for h in (h0, h1):
                        # offload some eb multiplies to GpSimd (slow but
                        # otherwise idle); pair 3's must stay on DVE so the
                        # tail's attnV is not gated by a 4us GpSimd op
                        eng = nc.gpsimd if (mp == 3 and j < 3) else nc.vector
                        eng.tensor_mul(
                            pt_tiles[(h, mp)][:], pt_tiles[(h, mp)][:],
                            eb_tiles[(h, mp)][:],
                        )
            if j == 0:
                vproj(m)
                if m == 1:
                    qkproj(1)
                if m == 3:
                    qkproj(2)
                if m == 5:
                    qkproj(3)

        # ---- normalization of pair j-1 ----
        if 1 <= j < NP:
            norm_folded(g0, g1)
        elif j == NP:
            norm_tail(g0, g1)

    # ---- output projection wave B: chunks kc=2,3 + partials, DMA out ----
    for t_i in range(MT):
        ps = app.tile([128, C], F32, tag="ap", name=f"oB{t_i}")
        for kc in (2, 3):
            nc.tensor.matmul(
                ps,
                ao_s[:, kc, t_i * 128:(t_i + 1) * 128],
                w_s["o"][:, kc, :],
                start=(kc == 2),
                stop=(kc == 3),
            )
        ot = sml.tile([128, C], F32, tag="ot")
        nc.vector.tensor_add(ot, ps, prt_tiles[t_i])
        nc.sync.dma_start(out=out[t_i * 128:(t_i + 1) * 128, :], in_=ot)


_CACHE = {}


def build_nc():
    if "nc" in _CACHE:
        return _CACHE["nc"]
    nc = bacc.Bacc(
        "TRN2", target_bir_lowering=False, debug=False, num_devices=NCORES
    )
    xT = nc.dram_tensor("xT", [128, KC, T], BF, kind="ExternalInput")
    wqT = nc.dram_tensor("wqT", [128, KC, C], BF, kind="ExternalInput")
    wkT = nc.dram_tensor("wkT", [128, KC, C], BF, kind="ExternalInput")
    wvT = nc.dram_tensor("wvT", [128, KC, C], BF, kind="ExternalInput")
    woT = nc.dram_tensor("woT", [128, KC, C], BF, kind="ExternalInput")
    ebT = nc.dram_tensor("ebT", [H, T, T], BF, kind="ExternalInput")
    bvec = nc.dram_tensor("bvec", [4, C], F32, kind="ExternalInput")
    out = nc.dram_tensor("out", [T, C], F32, kind="ExternalOutput")
    with tile.TileContext(nc) as tc:
        with ExitStack() as ctx:
            _body(ctx, tc, xT[:], wqT[:], wkT[:], wvT[:], woT[:], ebT[:], bvec[:], out[:])
    nc.compile()
    _CACHE["nc"] = nc
    return nc


def make_in_maps(inputs):
    x = np.asarray(inputs["x"], dtype=np.float32)
    attn_bias = np.asarray(inputs["attn_bias"], dtype=np.float32)
    mask = np.asarray(inputs["key_padding_mask"]).astype(bool)
    Wq = np.asarray(inputs["Wq"], dtype=np.float32)
    Wk = np.asarray(inputs["Wk"], dtype=np.float32)
    Wv = np.asarray(inputs["Wv"], dtype=np.float32)
    Wo = np.asarray(inputs["Wo"], dtype=np.float32)
    bq = np.asarray(inputs["bq"], dtype=np.float32)
    bk = np.asarray(inputs["bk"], dtype=np.float32)
    bv = np.asarray(inputs["bv"], dtype=np.float32)
    bo = np.asarray(inputs["bo"], dtype=np.float32)

    def pack(wT):
        # [C_in, C_out] -> SBUF image [128, KC, C_out] (partition p holds
        # rows {kc*128+p}) so the whole tensor is one DMA of 128
        # contiguous descriptors
        return np.ascontiguousarray(
            wT.reshape(KC, 128, wT.shape[1]).transpose(1, 0, 2)
        ).astype(BF_NP)

    scale = math.sqrt(D)
    wqT = pack((Wq / scale).T)
    wkT = pack(Wk.T)
    wvT = pack(Wv.T)
    woT = pack(Wo.T)
    bvec = np.stack([bq / scale, bk, bv, bo]).astype(np.float32)

    in_maps = []
    for b in range(B):
        xT = pack(x[b].T)
        ebT = np.exp(attn_bias[b].transpose(0, 2, 1))
        ebT[:, mask[b], :] = 0.0
        ebT = ebT.astype(BF_NP)
        in_maps.append(
            {
                "xT": xT,
                "wqT": wqT,
                "wkT": wkT,
                "wvT": wvT,
                "woT": woT,
                "ebT": ebT,
                "bvec": bvec,
            }
        )
    return in_maps


def run(inputs, trace=False):
    nc = build_nc()
    in_maps = make_in_maps(inputs)
    res = run_bass_kernel_spmd(nc, in_maps, list(range(NCORES)), trace=trace)
    out = np.stack(
        [np.asarray(res.results[i]["out"], dtype=np.float32) for i in range(B)], axis=0
    )
    return out, res.exec_time_ns


def kernel(**inputs):
    out, _ = run(inputs, trace=False)
    return out
